# revision 53
# baseline (speedup 1.0000x reference)
"""Trainium2 Bass kernel for nn_DSQGAttentionQW (sparse offset attention).

Sharding: pure head-tensor-parallel, zero collectives. Each core computes its
head's attention over all rows plus its head's partial output projection
(contract over that head's 64 channels of W_out); the host unshard sums the
8 partial [N, D] outputs (standard TP partial-sum gather) and adds b_out.
No sync point between cores -> per-core span is launch-skew free.
"""
import math
import numpy as np

import concourse.bacc as bacc
import concourse.bass as bass
import concourse.tile as tile
import concourse.mybir as mybir
import concourse.masks as masks
from concourse.bass_utils import run_bass_kernel_spmd

# ---- problem constants (must match reference.py) ----
_DENSE_LOCAL_W = 32
_DYADIC = [48, 64, 96, 128, 192, 256, 384, 512, 768, 1024, 1536, 2048, 3072, 4096]
OFFSETS = np.array(
    sorted(set(range(0, _DENSE_LOCAL_W + 1)) | set(_DYADIC)), dtype=np.int32
)  # [47]
NUM_OFFSETS = len(OFFSETS)
H = 8
_LOG_MAX = math.log(1.0 + 4096.0)
_HEAD_OMEGAS = [0.0, 0.0, 1 * math.pi / _LOG_MAX, 1 * math.pi / _LOG_MAX,
                4 * math.pi / _LOG_MAX, 4 * math.pi / _LOG_MAX,
                6 * math.pi / _LOG_MAX, 6 * math.pi / _LOG_MAX]
_log_d = np.log(1.0 + OFFSETS.astype(np.float64))
DISP_COS_KERNEL = np.zeros((NUM_OFFSETS, H), dtype=np.float32)
for _h, _om in enumerate(_HEAD_OMEGAS):
    if _om > 0.0:
        DISP_COS_KERNEL[:, _h] = np.cos(_om * _log_d)

B, N, D = 1, 2048, 512
HD = D // H
NC = 8
NT = N // 128           # 16 q-tiles of 128
# Effective k-tile depths m (delta in (128(m-1), 128m]) that can be causal for
# N=2048: depths 16/24/32 (delta >= 2048) are never valid.
R_DEPTHS = [0, 1, 2, 3, 4, 6, 8, 12]
NR = len(R_DEPTHS)

FP = mybir.dt.float32
FR = mybir.dt.float32r
F16 = mybir.dt.float16
F8 = mybir.dt.float8e4
# host-side pre-scales so fp8 e4m3 weights sit in the normal range;
# undone on-device in the PSUM->SBUF bias/activation copies.
SQ, SK, SV, SG = 512.0, 64.0, 64.0, 64.0

_cache = {}


def _build_masks(eff_pb_h: np.ndarray) -> np.ndarray:
    """maskW[ri, kp, i] = exp(eff_pb[offset_idx(delta)]) if delta valid else 0,
    with delta = i - kp + 128*m for depth m = R_DEPTHS[ri]."""
    off_idx = {int(d): i for i, d in enumerate(OFFSETS)}
    kp = np.arange(128)[None, :, None]
    i = np.arange(128)[None, None, :]
    m = np.array(R_DEPTHS)[:, None, None]
    delta = i - kp + 128 * m  # [NR, 128, 128]
    w = np.zeros((NR, 128, 128), dtype=np.float32)
    for d, oi in off_idx.items():
        sel = delta == d
        if sel.any():
            w[sel] = math.exp(float(eff_pb_h[oi]))
    return w


def _build_module():
    nc = bacc.Bacc("TRN2", target_bir_lowering=False, debug=False, num_devices=NC)

    xT = nc.dram_tensor("xT", [D, N], F16, kind="ExternalInput").ap()
    wA = nc.dram_tensor("wA", [D, 128], F16, kind="ExternalInput").ap()  # [Wq|Wk]
    wB = nc.dram_tensor("wB", [D, 128], F16, kind="ExternalInput").ap()  # [Wv|Wg]
    bA = nc.dram_tensor("bA", [128], FP, kind="ExternalInput").ap()
    bB = nc.dram_tensor("bB", [128], FP, kind="ExternalInput").ap()
    maskW = nc.dram_tensor("maskW", [128, NR, 128], F16, kind="ExternalInput").ap()
    woutP = nc.dram_tensor("woutP", [128, D], F16, kind="ExternalInput").ap()
    yout = nc.dram_tensor("y", [N, D], F16, kind="ExternalOutput").ap()

    with tile.TileContext(nc) as tc:
        with (
            tc.tile_pool(name="singles", bufs=1) as S,
            tc.tile_pool(name="work", bufs=4) as W,
            tc.tile_pool(name="pk", bufs=4) as PK,
            tc.tile_pool(name="ps", bufs=1, space="PSUM") as PS,
            tc.tile_pool(name="ps3", bufs=3, space="PSUM") as PS3,
            tc.tile_pool(name="pso", bufs=1, space="PSUM") as PSO,
        ):
            # ---------- PE warm-up (HAM) during initial DMA window ----------
            wconst = S.tile([128, 512], F16)
            nc.vector.memset(wconst[:], 1.0)
            psW = PS3.tile([128, 512], FP, tag="s")
            for _ in range(6):
                nc.tensor.matmul(psW[:], wconst[:, 0:128], wconst[:],
                                 start=True, stop=True)
            # prime the sigmoid act table first so identity/sigmoid (MM-A)
            # need no second load; the exp table load happens once at the
            # start of attention.
            tprime = W.tile([64, 1], F16, tag="tp")
            nc.scalar.activation(tprime[:], wconst[0:64, 0:1],
                                 mybir.ActivationFunctionType.Sigmoid)

            # ---------- constants / loads ----------
            ident = S.tile([128, 128], F16)
            masks.make_identity(nc, ident[:])

            wAs = S.tile([128, 4, 128], F16)
            nc.sync.dma_start(out=wAs[:], in_=wA.rearrange("(ct p) o -> p ct o", p=128))
            wBs = S.tile([128, 4, 128], F16)
            nc.sync.dma_start(out=wBs[:], in_=wB.rearrange("(ct p) o -> p ct o", p=128))
            bAs = S.tile([128, 1], FP)
            nc.sync.dma_start(out=bAs[:], in_=bA[:, None])
            bBs = S.tile([128, 1], FP)
            nc.sync.dma_start(out=bBs[:], in_=bB[:, None])

            # x^T in 2 independent 1024-col tiles (2KB DRAM lines = full DMA
            # rate) so MM-A can start once the first half lands.
            xT_r = xT.rearrange("(ct p) n -> p ct n", p=128)
            xss = []
            for half in range(2):
                xs_c = S.tile([128, 4, 1024], F16, tag=f"xs{half}")
                nsl = slice(1024 * half, 1024 * (half + 1))
                for ct in range(4):
                    nc.sync.dma_start(out=xs_c[:, ct, :], in_=xT_r[:, ct, nsl])
                xss.append(xs_c)

            # needed only from the attention phase on; issued after x loads
            mws = S.tile([128, NR, 128], F16)
            nc.sync.dma_start(out=mws[:], in_=maskW[:])
            woPs = S.tile([128, D], F16)
            nc.sync.dma_start(out=woPs[:], in_=woutP[:])

            # ---------- MM-A: qT / kT / vT / gT ----------
            # qT/kT zero-padded to 128 partitions: contract-128 matmuls
            # stream at ~2x the rate of contract-64 on TRN2 PE.
            qT = S.tile([128, N], F16)      # rows 0:64 q (pre-scaled), 64:128 zero
            nc.gpsimd.memset(qT[HD:128, :], 0.0)
            kT = S.tile([128, N], F16)
            nc.gpsimd.memset(kT[HD:128, :], 0.0)
            vT = S.tile([64, N], F16)       # if_gain folded
            gT = S.tile([HD + 1, N], FP)   # sigmoid gate; row 64 = 1.0 (denom)
            nc.vector.memset(gT[HD:HD + 1, :], 1.0)
            Vn = S.tile([128, NT, HD + 1], F16)
            nc.vector.memset(Vn[:, :, HD:HD + 1], 1.0)

            def emit_transpose(t):
                psT = PS3.tile([128, 64], F16, tag="s")
                nc.tensor.transpose(psT[:], vT[:, 128 * t:128 * (t + 1)],
                                    ident[0:64, 0:64])
                nc.vector.tensor_copy(Vn[:, t, 0:HD], psT[:])

            for nch in range(4):
                nsl = slice(512 * nch, 512 * (nch + 1))
                psA = PS.tile([128, 512], FP, tag="mma")
                psB = PS3.tile([128, 512], FP, tag="s")
                xsl = slice(512 * (nch % 2), 512 * (nch % 2) + 512)
                for ct in range(4):
                    nc.tensor.matmul(psA[:], wAs[:, ct, :],
                                     xss[nch // 2][:, ct, xsl],
                                     start=(ct == 0), stop=(ct == 3))
                for ct in range(4):
                    nc.tensor.matmul(psB[:], wBs[:, ct, :],
                                     xss[nch // 2][:, ct, xsl],
                                     start=(ct == 0), stop=(ct == 3))
                # biased copies: q,k on DVE; v,gate on ACT
                nc.vector.tensor_scalar_add(qT[0:HD, nsl], psA[0:64, :],
                                            bAs[0:64])
                nc.vector.tensor_scalar_add(kT[0:HD, nsl], psA[64:128, :],
                                            bAs[64:128])
                nc.scalar.activation(vT[:, nsl], psB[0:64, :],
                                     mybir.ActivationFunctionType.Identity,
                                     bias=bBs[0:64], scale=1.0)
                nc.scalar.activation(gT[0:HD, nsl], psB[64:128, :],
                                     mybir.ActivationFunctionType.Sigmoid,
                                     bias=bBs[64:128], scale=1.0)
                if nch >= 1:
                    for t in range(4 * (nch - 1), 4 * nch):
                        emit_transpose(t)

            for t in range(12, 16):
                emit_transpose(t)

            # ---------- attention (k-tile-major, fp16) ----------
            # out2_all: [65, N] PSUM accumulator (4 banks). Zero-init via K=1
            # start=True matmuls so subsequent MMs accumulate via has_written.
            out2_all = PSO.tile([HD + 1, N], FP)
            zrow = S.tile([1, 512], F16)
            nc.vector.memset(zrow[:], 0.0)
            z65 = S.tile([1, HD + 1], F16)
            nc.vector.memset(z65[:], 0.0)
            for c in range(4):
                nc.tensor.matmul(out2_all[:, 512 * c:512 * (c + 1)], z65[:],
                                 zrow[:], start=True, stop=False)

            # zX rows 0:64 gated attn out^T, row 64 denom, rows 65:128 zero
            # (zero-padded so MM3 runs at contract-128 rate; woutP rows 64:128
            # are zero host-side so the pad contributes nothing).
            zX = S.tile([128, N], F16)
            nc.gpsimd.memset(zX[HD:128, :], 0.0)

            Pks = [None] * NT

            def emit_mm1(tk):
                ksl = slice(128 * tk, 128 * (tk + 1))
                groups = []
                nq_near = min(3, NT - tk)
                groups.append(([0, 1, 2][:nq_near], tk, nq_near))
                nq34 = max(0, min(2, NT - tk - 3))
                if nq34:
                    groups.append(([3, 4][:nq34], tk + 3, nq34))
                if tk + 8 < NT:
                    groups.append(([6, 8], tk + 6, 2))
                elif tk + 6 < NT:
                    groups.append(([6], tk + 6, 1))
                if tk + 12 < NT:
                    groups.append(([12], tk + 12, 1))
                Pk = PK.tile([128, NR, 128], F16, tag="Pk")
                Pks[tk] = Pk
                for ms, q_lo, nq in groups:
                    if ms == [6, 8]:
                        # two q-tiles two apart: strided rhs, one matmul
                        q = qT[:]
                        qap = bass.AP(tensor=q.tensor,
                                      offset=q.offset + 128 * q_lo,
                                      ap=[q.ap[0], [256, 2], [1, 128]])
                    else:
                        qap = qT[:, 128 * q_lo:128 * (q_lo + nq)]
                    psS = PS3.tile([128, 384], FP, tag="s")
                    nc.tensor.matmul(psS[:, 0:128 * nq], kT[:, ksl],
                                     qap, start=True, stop=True)
                    expS = W.tile([128, 384], F16, tag="expS")
                    nc.scalar.activation(expS[:, 0:128 * nq], psS[:, 0:128 * nq],
                                         mybir.ActivationFunctionType.Exp)
                    ris = [R_DEPTHS.index(m) for m in ms]
                    if ris[0] >= 5:
                        nc.gpsimd.tensor_mul(
                            Pk[:, ris[0]:ris[0] + len(ris), :],
                            expS[:, 0:128 * len(ris)],
                            mws[:, ris[0]:ris[0] + len(ris), :])
                    else:
                        nc.vector.tensor_mul(
                            Pk[:, ris[0]:ris[0] + len(ris), :],
                            expS[:, 0:128 * len(ris)],
                            mws[:, ris[0]:ris[0] + len(ris), :])

            def emit_mm2(tk):
                Pk = Pks[tk]
                nq1 = min(4, NT - tk)
                nc.tensor.matmul(
                    out2_all[:, 128 * tk:128 * (tk + nq1)],
                    Vn[:, tk, :], Pk[:, 0:nq1, :],
                    start=False, stop=(tk == NT - 1), skip_group_check=True)
                # depths 4/6/8 write q-tiles tk+4/6/8: batch as one matmul
                # with a stride-256 free dim on the PSUM out.
                nd = sum(1 for m in (4, 6, 8) if tk + m < NT)
                if nd:
                    b = out2_all[:]
                    o2 = bass.AP(tensor=b.tensor,
                                 offset=b.offset + 128 * (tk + 4),
                                 ap=[b.ap[0], [256, nd], [1, 128]])
                    nc.tensor.matmul(o2, Vn[:, tk, :], Pk[:, 4:4 + nd, :],
                                     start=False, stop=False,
                                     skip_group_check=True)
                if tk + 12 < NT:
                    nc.tensor.matmul(
                        out2_all[:, 128 * (tk + 12):128 * (tk + 13)],
                        Vn[:, tk, :], Pk[:, 7, :],
                        start=False, stop=False, skip_group_check=True)

            def emit_epilogue(c, w=512):
                csl = slice(w * c, w * (c + 1))
                nc.vector.tensor_mul(zX[0:HD + 1, csl], out2_all[:, csl],
                                     gT[:, csl])

            def emit_mm3(j):
                # row-tile j: y[128j:128j+128, :] = (zX^T @ Wo_head) / denom
                sl = slice(128 * j, 128 * (j + 1))
                psD = PS3.tile([128, 64], F16, tag="s")
                nc.tensor.transpose(psD[:, 0:1], zX[HD:HD + 1, sl],
                                    ident[HD:HD + 1, HD:HD + 1])
                rd = W.tile([128, 1], FP, tag="rd")
                nc.vector.reciprocal(rd[:], psD[:, 0:1])
                psY = PS.tile([128, D], FP, tag="mma")
                nc.tensor.matmul(psY[:], zX[:, sl], woPs[:],
                                 start=True, stop=True)
                ysb = W.tile([128, D], F16, tag="ysb")
                if j % 2 == 0:
                    nc.vector.tensor_scalar_mul(ysb[:], psY[:], rd[:])
                else:
                    # identity shares the act table with exp -> no reload
                    nc.scalar.activation(ysb[:], psY[:],
                                         mybir.ActivationFunctionType.Identity,
                                         scale=rd[:])
                nc.sync.dma_start(out=yout[sl, :], in_=ysb[:])

            # chunks 0-2 epilogue at 512-col granularity; chunk 3 per
            # row-tile right after its last MM2 write, to shorten the tail.
            mm3_q = []
            for tk in range(NT):
                emit_mm1(tk)
                if tk >= 1:
                    emit_mm2(tk - 1)
                if tk in (4, 8, 12):
                    emit_epilogue(tk // 4 - 1)
                    mm3_q.extend(range(4 * (tk // 4 - 1), 4 * (tk // 4)))
                if tk >= 14:
                    emit_epilogue(tk - 2, w=128)   # row-tile 12 at tk=14, ...
                    mm3_q.append(tk - 2)
                if mm3_q:
                    emit_mm3(mm3_q.pop(0))
                if len(mm3_q) > 2:
                    emit_mm3(mm3_q.pop(0))
            emit_mm2(NT - 1)
            for j in (14, 15):
                emit_epilogue(j, w=128)
                mm3_q.append(j)
            for j in mm3_q:
                emit_mm3(j)

    nc.compile()
    return nc


def _prep_inputs(x, W_qkv, b_qkv, W_out, b_out, W_gate, b_gate,
                 pos_bias, scale_embed, if_gain, disp_amp):
    assert not np.any(np.asarray(scale_embed)), \
        "kernel fast path requires scale_embed == 0"
    xTn = np.ascontiguousarray(np.asarray(x)[0].T.astype(np.float32))  # [D, N]
    W_qkv = np.asarray(W_qkv, dtype=np.float32)
    b_qkv = np.asarray(b_qkv, dtype=np.float32)
    W_gate = np.asarray(W_gate, dtype=np.float32)
    b_gate = np.asarray(b_gate, dtype=np.float32)
    W_out = np.asarray(W_out, dtype=np.float32)
    pos_bias = np.asarray(pos_bias, dtype=np.float32)
    if_gain = np.asarray(if_gain, dtype=np.float32)
    disp_amp = np.asarray(disp_amp, dtype=np.float32)

    scl = 1.0 / math.sqrt(HD)
    xT16 = xTn.astype(np.float16)

    in_maps = []
    for h in range(NC):
        qs = slice(HD * h, HD * (h + 1))
        ks = slice(D + HD * h, D + HD * (h + 1))
        vs = slice(2 * D + HD * h, 2 * D + HD * (h + 1))
        wq = W_qkv[:, qs] * scl
        wk = W_qkv[:, ks]
        wv = W_qkv[:, vs] * if_gain[h]
        wg = W_gate[:, qs]
        bq = b_qkv[qs] * scl
        bk = b_qkv[ks]
        bv = b_qkv[vs] * if_gain[h]
        bg = b_gate[qs]
        eff_pb_h = pos_bias[:, h] + DISP_COS_KERNEL[:, h] * disp_amp[h]
        in_maps.append({
            "xT": xT16,
            "wA": np.ascontiguousarray(
                np.concatenate([wq, wk], axis=1)).astype(np.float16),
            "wB": np.ascontiguousarray(
                np.concatenate([wv, wg], axis=1)).astype(np.float16),
            "bA": np.ascontiguousarray(np.concatenate([bq, bk])),
            "bB": np.ascontiguousarray(np.concatenate([bv, bg])),
            "maskW": np.ascontiguousarray(
                _build_masks(eff_pb_h).transpose(1, 0, 2)).astype(np.float16),
            "woutP": np.concatenate(
                [W_out[qs, :], np.zeros((128 - HD, D), np.float32)],
                axis=0).astype(np.float16),
        })
    return in_maps


def _gather(res, b_out):
    """Unshard: sum the 8 per-head partial projections, add output bias."""
    acc = np.zeros((N, D), dtype=np.float32)
    for c in range(NC):
        acc += res.results[c]["y"].astype(np.float32)
    acc += np.asarray(b_out, dtype=np.float32)[None, :]
    return acc.reshape(B, N, D)


def kernel(**inputs) -> np.ndarray:
    if "nc" not in _cache:
        _cache["nc"] = _build_module()
    nc = _cache["nc"]
    in_maps = _prep_inputs(**inputs)
    res = run_bass_kernel_spmd(nc, in_maps, core_ids=list(range(NC)))
    return _gather(res, inputs["b_out"])
